# revision 8
# baseline (speedup 1.0000x reference)
"""Single-query cross-attention (B=16, S=4096, D=1024, H=16) on 8 TRN2 cores.

Math fold: for query length 1,
    scores[b,h,s] = (Wk_h^T q_h[b]) . enc[b,s,:] / sqrt(hd)   (q-tilde trick)
    ctx[b,h,:]    = Wv_h @ (sum_s w[b,h,s] enc[b,s,:])        (Wv fold)
so the big K/V projections (275 GFLOP) are never materialized; the kernel
streams encoder_outputs once per layout (memory bound).  Batch is sharded
2-per-core; no collectives.  Host-side prep is layout/dtype only (no math):
bf16/fp8 casts, weight transposes, and a second transposed copy of enc so the
scores contraction (over d) never needs an on-chip transpose.

v2 over the first working version:
  - natural-layout enc (the c-tilde side) is fp8e4m3: 24 MB HBM/core
    instead of 32; the ctx matmul consumes it mixed-dtype against bf16
    attention weights (w stays bf16 -- fp8 w fails the accuracy gate).
  - prologue restructured: dhT + per-jb weight slices DMA first and the
    q / q-tilde matmuls consume each 128-row slice as it lands; q-tildeT
    is produced directly by a block-diagonal-masked full contraction
    (the zero blocks do the per-head masking), killing two transpose
    round-trips off the serial startup chain.
  - epilogue: one batched [64,1024] matmul against WvT + 32 select
    copies instead of 128 tiny per-head matmuls.
"""

import sys
import numpy as np

for _p in ("/opt/trn_rl_repo",):
    if _p not in sys.path:
        sys.path.insert(0, _p)

import ml_dtypes
import concourse.bass as bass
import concourse.bacc as bacc
import concourse.tile as tile
from concourse import mybir
from concourse.masks import make_identity
from concourse.bass_utils import run_bass_kernel_spmd

B, S, D, H = 16, 4096, 1024, 16
HD = D // H                      # 64
NCORES = 8
BPC = B // NCORES                # 2 batches per core
NJ = D // 128                    # 8 d-blocks
GRP = 4                          # s-tiles per scores group (512 cols)
SQ = 1024                        # encT s-quarter width

F32 = mybir.dt.float32
BF16 = mybir.dt.bfloat16
FP8 = mybir.dt.float8e4
USE_FP8_C = True          # natural enc (c-tilde rhs) in fp8e4m3; w stays bf16
EDT = FP8 if USE_FP8_C else BF16
EXP_BIAS = -2.0           # exp(s-2); cancels in 1/l


def build_nc(s=S):
    nc = bacc.Bacc(None, target_bir_lowering=False, debug=False)

    dhT_ext = nc.declare_dram_parameter("dhT", [D, BPC], BF16, isOutput=False)
    enc_ext = nc.declare_dram_parameter("enc", [BPC, s, D], EDT, isOutput=False)
    encT_ext = nc.declare_dram_parameter("encT", [BPC, D, s], BF16, isOutput=False)
    wqT_ext = nc.declare_dram_parameter("wqT", [D, D], BF16, isOutput=False)
    wk_ext = nc.declare_dram_parameter("wk", [D, D], BF16, isOutput=False)
    wvT_ext = nc.declare_dram_parameter("wvT", [D, D], BF16, isOutput=False)
    out_ext = nc.declare_dram_parameter("out", [BPC, D], F32, isOutput=True)

    with tile.TileContext(nc) as tc:
        _build(nc, tc, s, dhT_ext, enc_ext, encT_ext, wqT_ext, wk_ext, wvT_ext, out_ext)
    nc.compile()
    return nc


def _build(nc, tc, s, dhT_ext, enc_ext, encT_ext, wqT_ext, wk_ext, wvT_ext, out_ext):
    NT = s // 128                # s-tiles per batch
    NG = NT // GRP               # scores groups per batch
    NQ = max(1, s // SQ)         # encT quarters per batch
    GPQ = NG // NQ               # scores groups per quarter
    from contextlib import ExitStack

    ctx = ExitStack()
    with ctx:
        singles = ctx.enter_context(tc.tile_pool(name="singles", bufs=1))
        wjb = ctx.enter_context(tc.tile_pool(name="wjb", bufs=1))
        wq_enc = ctx.enter_context(tc.tile_pool(name="wq_enc", bufs=4))
        epool = ctx.enter_context(tc.tile_pool(name="epool", bufs=30))
        sc = ctx.enter_context(tc.tile_pool(name="sc", bufs=2))
        wts = ctx.enter_context(tc.tile_pool(name="wts", bufs=10))
        pp_bf = ctx.enter_context(tc.tile_pool(name="pp_bf", bufs=2, space="PSUM"))
        pp_f32 = ctx.enter_context(tc.tile_pool(name="pp_f32", bufs=2, space="PSUM"))
        pp_sc = ctx.enter_context(tc.tile_pool(name="pp_sc", bufs=2, space="PSUM"))

        # ---- weights/inputs: dhT first (tiny, unblocks q matmuls), then
        # per-jb slices of WqT and Wk so the prologue pipelines with DMA.
        dhT_sb = singles.tile([128, NJ, BPC], BF16, tag="dhT")
        nc.sync.dma_start(
            out=dhT_sb, in_=dhT_ext[:, :].rearrange("(jb p) b -> p jb b", p=128)
        )
        wq_jb = []
        wk_jb = []
        for jb in range(NJ):
            t = wjb.tile([128, D], BF16, tag=f"wq{jb}")
            nc.sync.dma_start(out=t, in_=wqT_ext[jb * 128:(jb + 1) * 128, :])
            wq_jb.append(t)
        for jb in range(NJ):
            t = wjb.tile([128, D], BF16, tag=f"wk{jb}")
            nc.sync.dma_start(out=t, in_=wk_ext[jb * 128:(jb + 1) * 128, :])
            wk_jb.append(t)

        # ---- constants
        ident = singles.tile([128, 128], BF16)
        make_identity(nc, ident)

        # ---- q[b, i] = sum_d dh[b, d] Wq[i, d]; consume wq slices as they land
        q_ps = pp_f32.tile([BPC, D], F32, tag="pf32")
        for jb in range(NJ):
            for chunk in range(2):
                cs = slice(chunk * 512, (chunk + 1) * 512)
                nc.tensor.matmul(
                    q_ps[:, cs],
                    dhT_sb[:, jb, :],
                    wq_jb[jb][:, cs],
                    start=(jb == 0),
                    stop=(jb == NJ - 1),
                    skip_group_check=True,
                )
        q_sb = singles.tile([BPC, D], BF16, tag="q")
        nc.vector.tensor_copy(out=q_sb, in_=q_ps)

        # qhT: block-diagonal [i, r], r = h*2 + b; qhT[i, r] = q[b, i] iff head(i)==h.
        # Built from 8 PE transposes of q; each 128-row block covers heads 2jb, 2jb+1.
        qhT_sb = singles.tile([128, NJ, 2 * H], BF16, tag="qhT")
        nc.vector.memset(qhT_sb, 0.0)
        for jb in range(NJ):
            ps = pp_bf.tile([128, 128], BF16, tag="ppsum_big")
            nc.tensor.transpose(
                ps[:, 0:BPC], q_sb[:, jb * 128:(jb + 1) * 128], ident[0:BPC, 0:BPC]
            )
            nc.vector.tensor_copy(
                out=qhT_sb[0:64, jb, 4 * jb:4 * jb + 2], in_=ps[0:64, 0:BPC]
            )
            nc.vector.tensor_copy(
                out=qhT_sb[64:128, jb, 4 * jb + 2:4 * jb + 4], in_=ps[64:128, 0:BPC]
            )

        # q-tildeT directly: qtT[d', r] = sum_i Wk[i, d'] qhT[i, r]; the
        # block-diagonal zeros in qhT mask the contraction to head(i)==h(r).
        qtT_all = singles.tile([128, NJ, 2 * H], BF16, tag="qtT_all")
        for db in range(NJ):
            qt_ps = pp_sc.tile([128, 2 * H], F32, tag="s_ps")
            for jb_i in range(NJ):
                nc.tensor.matmul(
                    qt_ps,
                    wk_jb[jb_i][:, db * 128:(db + 1) * 128],
                    qhT_sb[:, jb_i, :],
                    start=(jb_i == 0),
                    stop=(jb_i == NJ - 1),
                )
            nc.vector.tensor_scalar_mul(
                qtT_all[:, db, :], qt_ps, 1.0 / np.sqrt(HD)
            )
        qtT_b = []
        qtT_v = qtT_all.rearrange("p j (h b) -> p j h b", b=BPC)
        for b in range(BPC):
            t = singles.tile([128, NJ, H], BF16, tag=f"qtT{b}")
            nc.vector.tensor_copy(out=t, in_=qtT_v[:, :, :, b])
            qtT_b.append(t)

        # ---- main streaming loop (single pass, unnormalized-exp softmax)
        cmerged = singles.tile([64, D], BF16, tag="cmerged")
        nc.vector.memset(cmerged, 0.0)
        ebias = singles.tile([H, 1], F32, tag="ebias")
        nc.vector.memset(ebias, EXP_BIAS)
        sq = min(SQ, s)

        def load_etq(b, q):
            etq = wq_enc.tile([128, NJ, sq], BF16, tag="big")
            nc.sync.dma_start(
                out=etq,
                in_=encT_ext[b, :, q * sq:(q + 1) * sq].rearrange(
                    "(jb p) t -> p jb t", p=128
                ),
            )
            return etq

        etq_cur = []
        for b in range(BPC):
            etq_cur.append(load_etq(b, 0))
        etq_cur2 = list(etq_cur)
        lparts = []
        c_ps = []
        for b in range(BPC):
            lp = sc.tile([H, NG], F32, tag=f"lparts{b}")
            lparts.append(lp)
            cp = pp_f32.tile([H, D], F32, tag="pf32")
            c_ps.append(cp)
        wvT_sb = None
        for g in range(NG):
            for b in range(BPC):
                gg = g % GPQ
                if gg == 0 and g > 0:
                    etq_cur[b] = etq_cur2[b]
                # staggered one-quarter-ahead prefetch
                pre = g + (GPQ - b % GPQ)
                if gg == b % GPQ and pre // GPQ < NQ and pre // GPQ > 0:
                    etq_cur2[b] = load_etq(b, pre // GPQ)
                if g == NG - 2 and b == 0 and wvT_sb is None:
                    wvT_sb = singles.tile([128, NJ, D], BF16, tag="wvT")
                    nc.sync.dma_start(
                        out=wvT_sb,
                        in_=wvT_ext[:, :].rearrange("(jb p) d -> p jb d", p=128),
                    )
                etq = etq_cur[b]
                e_ts = []
                for tt in range(GRP):
                    t = g * GRP + tt
                    e_t = epool.tile([128, D], EDT, tag="e")
                    nc.sync.dma_start(
                        out=e_t, in_=enc_ext[b, t * 128:(t + 1) * 128, :]
                    )
                    e_ts.append(e_t)
                # scores for this group of 512 positions
                s_ps = pp_sc.tile([H, 512], F32, tag="s_ps")
                for jb in range(NJ):
                    nc.tensor.matmul(
                        s_ps,
                        qtT_b[b][:, jb, :],
                        etq[:, jb, gg * 512:(gg + 1) * 512],
                        start=(jb == 0),
                        stop=(jb == NJ - 1),
                    )
                # unnormalized weights (bf16), straight from PSUM
                w_g = sc.tile([H, 512], BF16, tag="w_g")
                nc.scalar.activation(
                    out=w_g,
                    in_=s_ps,
                    func=mybir.ActivationFunctionType.Exp,
                    bias=ebias,
                    accum_out=lparts[b][:, g:g + 1],
                )
                # wT tiles and c-tilde accumulation for the 4 s-tiles
                for tt in range(GRP):
                    ps = pp_bf.tile([128, 128], BF16, tag="ppsum_big")
                    nc.tensor.transpose(
                        ps[:, 0:H],
                        w_g[:, tt * 128:(tt + 1) * 128],
                        ident[0:H, 0:H],
                    )
                    wt_t = wts.tile([128, H], BF16, tag="wt")
                    nc.vector.tensor_copy(out=wt_t, in_=ps[:, 0:H])
                    t = g * GRP + tt
                    first = t == 0
                    last = t == NT - 1
                    for chunk in range(2):
                        cs = slice(chunk * 512, (chunk + 1) * 512)
                        nc.tensor.matmul(
                            c_ps[b][:, cs],
                            wt_t,
                            e_ts[tt][:, cs],
                            start=first,
                            stop=last,
                            skip_group_check=True,
                        )
        # normalize by 1/sum(exp) while copying out of PSUM
        for b in range(BPC):
            lsum = sc.tile([H, 1], F32, tag=f"lsum{b}")
            nc.vector.reduce_sum(lsum, lparts[b], axis=mybir.AxisListType.X)
            linv = sc.tile([H, 1], F32, tag=f"linv{b}")
            nc.vector.reciprocal(linv, lsum)
            nc.vector.tensor_scalar_mul(
                cmerged[b * 32:b * 32 + H, :], c_ps[b], linv
            )

        # ---- epilogue: cT then one batched matmul against WvT
        cT_sb = singles.tile([128, NJ, 64], BF16, tag="cT")
        for jb in range(NJ):
            ps = pp_bf.tile([128, 128], BF16, tag="ppsum_big")
            nc.tensor.transpose(
                ps[:, 0:64],
                cmerged[:, jb * 128:(jb + 1) * 128],
                ident[0:64, 0:64],
            )
            nc.vector.tensor_copy(out=cT_sb[:, jb, :], in_=ps[:, 0:64])

        # o[r'=(b*32+h), i] = sum_d cT[d, r'] WvT[d, i]; ctx[b, h*64+j] is the
        # (b*32+h, h*64+j) entry -- select the diagonal blocks afterwards.
        o_ps = pp_f32.tile([64, D], F32, tag="pf32")
        for chunk in range(2):
            cs = slice(chunk * 512, (chunk + 1) * 512)
            for jb in range(NJ):
                nc.tensor.matmul(
                    o_ps[:, cs],
                    cT_sb[:, jb, :],
                    wvT_sb[:, jb, cs],
                    start=(jb == 0),
                    stop=(jb == NJ - 1),
                    skip_group_check=True,
                )
        osb = singles.tile([64, D], F32, tag="osb")
        nc.vector.tensor_copy(out=osb, in_=o_ps)
        for h in range(H):
            hs = slice(h * HD, (h + 1) * HD)
            for b in range(BPC):
                r = b * 32 + h
                nc.sync.dma_start(
                    out=out_ext[b, hs], in_=osb[r:r + 1, hs]
                )


_NC_CACHE = None


def _get_nc():
    global _NC_CACHE
    if _NC_CACHE is None:
        _NC_CACHE = build_nc()
    return _NC_CACHE


def _shard(inputs):
    """Host-side prep: shard batch, cast dtypes, pre-transpose layouts."""
    bf = ml_dtypes.bfloat16
    dh = np.asarray(inputs["decoder_hidden"], dtype=np.float32)
    enc = np.asarray(inputs["encoder_outputs"], dtype=np.float32)
    wqT = np.ascontiguousarray(np.asarray(inputs["Wq"], dtype=np.float32).T).astype(bf)
    wk = np.ascontiguousarray(np.asarray(inputs["Wk"], dtype=np.float32)).astype(bf)
    wvT = np.ascontiguousarray(np.asarray(inputs["Wv"], dtype=np.float32).T).astype(bf)
    cdt = ml_dtypes.float8_e4m3 if USE_FP8_C else bf
    enc_c = enc.astype(cdt)
    in_maps = []
    for c in range(NCORES):
        sl = slice(c * BPC, (c + 1) * BPC)
        dhT = np.ascontiguousarray(dh[sl].T).astype(bf)
        eb = np.ascontiguousarray(enc_c[sl])
        ebT = np.ascontiguousarray(enc[sl].astype(bf).transpose(0, 2, 1))
        in_maps.append(
            {
                "dhT": dhT,
                "enc": eb,
                "encT": ebT,
                "wqT": wqT,
                "wk": wk,
                "wvT": wvT,
            }
        )
    return in_maps


def _run(inputs, trace=False, **kw):
    global _NC_CACHE
    in_maps = _shard(inputs)
    last_err = None
    for attempt in range(3):
        try:
            nc = _get_nc()
            res = run_bass_kernel_spmd(
                nc, in_maps, core_ids=list(range(NCORES)), trace=trace, **kw
            )
            out = np.concatenate([np.asarray(r["out"]) for r in res.results], axis=0)
            return out.astype(np.float32), res
        except Exception as e:  # transient NRT_EXEC_UNIT_UNRECOVERABLE etc.
            last_err = e
            _NC_CACHE = None  # rebuild the graph fresh on retry
            import time
            time.sleep(2.0)
    raise last_err


def kernel(**inputs):
    out, _ = _run(inputs, trace=False)
    return out


# revision 10
# speedup vs baseline: 1.0617x; 1.0617x over previous
"""Single-query cross-attention (B=16, S=4096, D=1024, H=16) on 8 TRN2 cores.

Math fold: for query length 1,
    scores[b,h,s] = (Wk_h^T q_h[b]) . enc[b,s,:] / sqrt(hd)   (q-tilde trick)
    ctx[b,h,:]    = Wv_h @ (sum_s w[b,h,s] enc[b,s,:])        (Wv fold)
so the big K/V projections (275 GFLOP) are never materialized; the kernel
streams encoder_outputs once per layout (memory bound).  Batch is sharded
2-per-core; no collectives.  Host-side prep is layout/dtype only (no math):
bf16/fp8 casts, weight transposes, and a second transposed copy of enc so the
scores contraction (over d) never needs an on-chip transpose.

v2 over the first working version:
  - natural-layout enc (the c-tilde side) is fp8e4m3: 24 MB HBM/core
    instead of 32; the ctx matmul consumes it mixed-dtype against bf16
    attention weights (w stays bf16 -- fp8 w fails the accuracy gate).
  - prologue restructured: dhT + per-jb weight slices DMA first and the
    q / q-tilde matmuls consume each 128-row slice as it lands; q-tildeT
    is produced directly by a block-diagonal-masked full contraction
    (the zero blocks do the per-head masking), killing two transpose
    round-trips off the serial startup chain.
  - epilogue: one batched [64,1024] matmul against WvT + 32 select
    copies instead of 128 tiny per-head matmuls.
"""

import sys
import numpy as np

for _p in ("/opt/trn_rl_repo",):
    if _p not in sys.path:
        sys.path.insert(0, _p)

import ml_dtypes
import concourse.bass as bass
import concourse.bacc as bacc
import concourse.tile as tile
from concourse import mybir
from concourse.masks import make_identity
from concourse.bass_utils import run_bass_kernel_spmd

B, S, D, H = 16, 4096, 1024, 16
HD = D // H                      # 64
NCORES = 8
BPC = B // NCORES                # 2 batches per core
NJ = D // 128                    # 8 d-blocks
GRP = 4                          # s-tiles per scores group (512 cols)
SQ = 1024                        # encT s-quarter width

F32 = mybir.dt.float32
BF16 = mybir.dt.bfloat16
FP8 = mybir.dt.float8e4
USE_FP8_C = True          # natural enc (c-tilde rhs) in fp8e4m3; w stays bf16
EDT = FP8 if USE_FP8_C else BF16
EXP_BIAS = -2.0           # exp(s-2); cancels in 1/l


def build_nc(s=S):
    nc = bacc.Bacc(None, target_bir_lowering=False, debug=False)

    dhT_ext = nc.declare_dram_parameter("dhT", [D, BPC], BF16, isOutput=False)
    enc_ext = nc.declare_dram_parameter("enc", [BPC, s, D], EDT, isOutput=False)
    encT_ext = nc.declare_dram_parameter("encT", [BPC, D, s], BF16, isOutput=False)
    wqT_ext = nc.declare_dram_parameter("wqT", [D, D], BF16, isOutput=False)
    wk_ext = nc.declare_dram_parameter("wk", [D, D], BF16, isOutput=False)
    wvT_ext = nc.declare_dram_parameter("wvT", [D, D], BF16, isOutput=False)
    out_ext = nc.declare_dram_parameter("out", [BPC, D], F32, isOutput=True)

    with tile.TileContext(nc) as tc:
        _build(nc, tc, s, dhT_ext, enc_ext, encT_ext, wqT_ext, wk_ext, wvT_ext, out_ext)
    nc.compile()
    return nc


def _build(nc, tc, s, dhT_ext, enc_ext, encT_ext, wqT_ext, wk_ext, wvT_ext, out_ext):
    NT = s // 128                # s-tiles per batch
    NG = NT // GRP               # scores groups per batch
    NQ = max(1, s // SQ)         # encT quarters per batch
    GPQ = NG // NQ               # scores groups per quarter
    from contextlib import ExitStack

    ctx = ExitStack()
    with ctx:
        singles = ctx.enter_context(tc.tile_pool(name="singles", bufs=1))
        wjb = ctx.enter_context(tc.tile_pool(name="wjb", bufs=1))
        wq_enc = ctx.enter_context(tc.tile_pool(name="wq_enc", bufs=5))
        epool = ctx.enter_context(tc.tile_pool(name="epool", bufs=30))
        sc = ctx.enter_context(tc.tile_pool(name="sc", bufs=2))
        wts = ctx.enter_context(tc.tile_pool(name="wts", bufs=10))
        pp_bf = ctx.enter_context(tc.tile_pool(name="pp_bf", bufs=2, space="PSUM"))
        pp_f32 = ctx.enter_context(tc.tile_pool(name="pp_f32", bufs=2, space="PSUM"))
        pp_sc = ctx.enter_context(tc.tile_pool(name="pp_sc", bufs=2, space="PSUM"))

        # ---- weights/inputs: dhT first (tiny, unblocks q matmuls), then
        # per-jb slices of WqT and Wk so the prologue pipelines with DMA.
        dhT_sb = singles.tile([128, NJ, BPC], BF16, tag="dhT")
        nc.sync.dma_start(
            out=dhT_sb, in_=dhT_ext[:, :].rearrange("(jb p) b -> p jb b", p=128)
        )
        wq_jb = []
        wk_jb = []
        for jb in range(NJ):
            t = wjb.tile([128, D], BF16, tag=f"wq{jb}")
            nc.sync.dma_start(out=t, in_=wqT_ext[jb * 128:(jb + 1) * 128, :])
            wq_jb.append(t)
        for jb in range(NJ):
            t = wjb.tile([128, D], BF16, tag=f"wk{jb}")
            nc.sync.dma_start(out=t, in_=wk_ext[jb * 128:(jb + 1) * 128, :])
            wk_jb.append(t)

        # ---- constants
        ident = singles.tile([128, 128], BF16)
        make_identity(nc, ident)

        # ---- q[b, i] = sum_d dh[b, d] Wq[i, d]; consume wq slices as they land
        q_ps = pp_f32.tile([BPC, D], F32, tag="pf32")
        for jb in range(NJ):
            for chunk in range(2):
                cs = slice(chunk * 512, (chunk + 1) * 512)
                nc.tensor.matmul(
                    q_ps[:, cs],
                    dhT_sb[:, jb, :],
                    wq_jb[jb][:, cs],
                    start=(jb == 0),
                    stop=(jb == NJ - 1),
                    skip_group_check=True,
                )
        q_sb = singles.tile([BPC, D], BF16, tag="q")
        nc.vector.tensor_copy(out=q_sb, in_=q_ps)

        # qhT: block-diagonal [i, r], r = h*2 + b; qhT[i, r] = q[b, i] iff head(i)==h.
        # Built from 8 PE transposes of q; each 128-row block covers heads 2jb, 2jb+1.
        qhT_sb = singles.tile([128, NJ, 2 * H], BF16, tag="qhT")
        nc.vector.memset(qhT_sb, 0.0)
        for jb in range(NJ):
            ps = pp_bf.tile([128, 128], BF16, tag="ppsum_big")
            nc.tensor.transpose(
                ps[:, 0:BPC], q_sb[:, jb * 128:(jb + 1) * 128], ident[0:BPC, 0:BPC]
            )
            nc.vector.tensor_copy(
                out=qhT_sb[0:64, jb, 4 * jb:4 * jb + 2], in_=ps[0:64, 0:BPC]
            )
            nc.vector.tensor_copy(
                out=qhT_sb[64:128, jb, 4 * jb + 2:4 * jb + 4], in_=ps[64:128, 0:BPC]
            )

        # q-tildeT directly: qtT[d', r] = sum_i Wk[i, d'] qhT[i, r]; the
        # block-diagonal zeros in qhT mask the contraction to head(i)==h(r).
        qtT_all = singles.tile([128, NJ, 2 * H], BF16, tag="qtT_all")
        for db in range(NJ):
            qt_ps = pp_sc.tile([128, 2 * H], F32, tag="s_ps")
            for jb_i in range(NJ):
                nc.tensor.matmul(
                    qt_ps,
                    wk_jb[jb_i][:, db * 128:(db + 1) * 128],
                    qhT_sb[:, jb_i, :],
                    start=(jb_i == 0),
                    stop=(jb_i == NJ - 1),
                )
            nc.vector.tensor_scalar_mul(
                qtT_all[:, db, :], qt_ps, 1.0 / np.sqrt(HD)
            )
        qtT_b = []
        qtT_v = qtT_all.rearrange("p j (h b) -> p j h b", b=BPC)
        for b in range(BPC):
            t = singles.tile([128, NJ, H], BF16, tag=f"qtT{b}")
            nc.vector.tensor_copy(out=t, in_=qtT_v[:, :, :, b])
            qtT_b.append(t)

        # ---- main streaming loop (single pass, unnormalized-exp softmax)
        cmerged = singles.tile([64, D], BF16, tag="cmerged")
        nc.vector.memset(cmerged, 0.0)
        ebias = singles.tile([H, 1], F32, tag="ebias")
        nc.vector.memset(ebias, EXP_BIAS)
        sq = min(SQ, s)

        def load_etq(b, q):
            etq = wq_enc.tile([128, NJ, sq], BF16, tag="big")
            nc.sync.dma_start(
                out=etq,
                in_=encT_ext[b, :, q * sq:(q + 1) * sq].rearrange(
                    "(jb p) t -> p jb t", p=128
                ),
            )
            return etq

        etq_cur = []
        for b in range(BPC):
            etq_cur.append(load_etq(b, 0))
        etq_cur2 = list(etq_cur)
        lparts = []
        c_ps = []
        for b in range(BPC):
            lp = sc.tile([H, NG], F32, tag=f"lparts{b}")
            lparts.append(lp)
            cp = pp_f32.tile([H, D], F32, tag="pf32")
            c_ps.append(cp)
        wvT_sb = None
        for g in range(NG):
            for b in range(BPC):
                gg = g % GPQ
                if gg == 0 and g > 0:
                    etq_cur[b] = etq_cur2[b]
                # staggered one-quarter-ahead prefetch
                pre = g + (GPQ - b % GPQ)
                if gg == b % GPQ and pre // GPQ < NQ and pre // GPQ > 0:
                    etq_cur2[b] = load_etq(b, pre // GPQ)
                if g == NG - 2 and b == 0 and wvT_sb is None:
                    wvT_sb = singles.tile([128, NJ, D], BF16, tag="wvT")
                    nc.sync.dma_start(
                        out=wvT_sb,
                        in_=wvT_ext[:, :].rearrange("(jb p) d -> p jb d", p=128),
                    )
                etq = etq_cur[b]
                e_ts = []
                for tt in range(GRP):
                    t = g * GRP + tt
                    e_t = epool.tile([128, D], EDT, tag="e")
                    nc.sync.dma_start(
                        out=e_t, in_=enc_ext[b, t * 128:(t + 1) * 128, :]
                    )
                    e_ts.append(e_t)
                # scores for this group of 512 positions
                s_ps = pp_sc.tile([H, 512], F32, tag="s_ps")
                for jb in range(NJ):
                    nc.tensor.matmul(
                        s_ps,
                        qtT_b[b][:, jb, :],
                        etq[:, jb, gg * 512:(gg + 1) * 512],
                        start=(jb == 0),
                        stop=(jb == NJ - 1),
                    )
                # unnormalized weights (bf16), straight from PSUM
                w_g = sc.tile([H, 512], BF16, tag="w_g")
                nc.scalar.activation(
                    out=w_g,
                    in_=s_ps,
                    func=mybir.ActivationFunctionType.Exp,
                    bias=ebias,
                    accum_out=lparts[b][:, g:g + 1],
                )
                # wT tiles and c-tilde accumulation for the 4 s-tiles
                for tt in range(GRP):
                    ps = pp_bf.tile([128, 128], BF16, tag="ppsum_big")
                    nc.tensor.transpose(
                        ps[:, 0:H],
                        w_g[:, tt * 128:(tt + 1) * 128],
                        ident[0:H, 0:H],
                    )
                    wt_t = wts.tile([128, H], BF16, tag="wt")
                    nc.vector.tensor_copy(out=wt_t, in_=ps[:, 0:H])
                    t = g * GRP + tt
                    first = t == 0
                    last = t == NT - 1
                    for chunk in range(2):
                        cs = slice(chunk * 512, (chunk + 1) * 512)
                        nc.tensor.matmul(
                            c_ps[b][:, cs],
                            wt_t,
                            e_ts[tt][:, cs],
                            start=first,
                            stop=last,
                            skip_group_check=True,
                        )
        # normalize by 1/sum(exp) while copying out of PSUM
        for b in range(BPC):
            lsum = sc.tile([H, 1], F32, tag=f"lsum{b}")
            nc.vector.reduce_sum(lsum, lparts[b], axis=mybir.AxisListType.X)
            linv = sc.tile([H, 1], F32, tag=f"linv{b}")
            nc.vector.reciprocal(linv, lsum)
            nc.vector.tensor_scalar_mul(
                cmerged[b * 32:b * 32 + H, :], c_ps[b], linv
            )

        # ---- epilogue: cT then one batched matmul against WvT
        cT_sb = singles.tile([128, NJ, 64], BF16, tag="cT")
        for jb in range(NJ):
            ps = pp_bf.tile([128, 128], BF16, tag="ppsum_big")
            nc.tensor.transpose(
                ps[:, 0:64],
                cmerged[:, jb * 128:(jb + 1) * 128],
                ident[0:64, 0:64],
            )
            nc.vector.tensor_copy(out=cT_sb[:, jb, :], in_=ps[:, 0:64])

        # ctx[b, h*64+j] = sum_d cT[d, b*32+h] WvT[d, h*64+j]
        ctx_ps = pp_f32.tile([BPC, D], F32, tag="pf32")
        cT_v = cT_sb.rearrange("p j (bb h) -> p j bb h", bb=BPC)
        for h in range(H):
            hs = slice(h * HD, (h + 1) * HD)
            for jb in range(NJ):
                nc.tensor.matmul(
                    ctx_ps[:, hs],
                    cT_v[:, jb, :, h],
                    wvT_sb[:, jb, hs],
                    start=(jb == 0),
                    stop=(jb == NJ - 1),
                )
        ob = singles.tile([BPC, D], F32, tag="out_sb")
        nc.vector.tensor_copy(out=ob, in_=ctx_ps)
        nc.sync.dma_start(out=out_ext[:, :], in_=ob)


_NC_CACHE = None


def _get_nc():
    global _NC_CACHE
    if _NC_CACHE is None:
        _NC_CACHE = build_nc()
    return _NC_CACHE


def _shard(inputs):
    """Host-side prep: shard batch, cast dtypes, pre-transpose layouts."""
    bf = ml_dtypes.bfloat16
    dh = np.asarray(inputs["decoder_hidden"], dtype=np.float32)
    enc = np.asarray(inputs["encoder_outputs"], dtype=np.float32)
    wqT = np.ascontiguousarray(np.asarray(inputs["Wq"], dtype=np.float32).T).astype(bf)
    wk = np.ascontiguousarray(np.asarray(inputs["Wk"], dtype=np.float32)).astype(bf)
    wvT = np.ascontiguousarray(np.asarray(inputs["Wv"], dtype=np.float32).T).astype(bf)
    cdt = ml_dtypes.float8_e4m3 if USE_FP8_C else bf
    enc_c = enc.astype(cdt)
    in_maps = []
    for c in range(NCORES):
        sl = slice(c * BPC, (c + 1) * BPC)
        dhT = np.ascontiguousarray(dh[sl].T).astype(bf)
        eb = np.ascontiguousarray(enc_c[sl])
        ebT = np.ascontiguousarray(enc[sl].astype(bf).transpose(0, 2, 1))
        in_maps.append(
            {
                "dhT": dhT,
                "enc": eb,
                "encT": ebT,
                "wqT": wqT,
                "wk": wk,
                "wvT": wvT,
            }
        )
    return in_maps


def _run(inputs, trace=False, **kw):
    global _NC_CACHE
    in_maps = _shard(inputs)
    last_err = None
    for attempt in range(3):
        try:
            nc = _get_nc()
            res = run_bass_kernel_spmd(
                nc, in_maps, core_ids=list(range(NCORES)), trace=trace, **kw
            )
            out = np.concatenate([np.asarray(r["out"]) for r in res.results], axis=0)
            return out.astype(np.float32), res
        except Exception as e:  # transient NRT_EXEC_UNIT_UNRECOVERABLE etc.
            last_err = e
            _NC_CACHE = None  # rebuild the graph fresh on retry
            import time
            time.sleep(2.0)
    raise last_err


def kernel(**inputs):
    out, _ = _run(inputs, trace=False)
    return out


# revision 16
# speedup vs baseline: 1.1871x; 1.1181x over previous
"""Single-query cross-attention (B=16, S=4096, D=1024, H=16) on 8 TRN2 cores.

Math fold: for query length 1,
    scores[b,h,s] = (Wk_h^T q_h[b]) . enc[b,s,:] / sqrt(hd)   (q-tilde trick)
    ctx[b,h,:]    = Wv_h @ (sum_s w[b,h,s] enc[b,s,:])        (Wv fold)
so the big K/V projections (275 GFLOP) are never materialized; the kernel
streams encoder_outputs once per layout.  Batch is sharded 2-per-core; no
collectives.  Host-side prep is layout/dtype only (no math): bf16/fp8
casts, weight transposes, a transposed copy of enc for the scores
contraction, and an s-permutation baked into both enc layouts (softmax and
the weighted sum are s-permutation invariant).

Key structure:
  - natural-layout enc (c-tilde side) is fp8e4m3, packed 4 s-rows per
    partition (4 KB DMA descriptors); the ctx matmul consumes it
    mixed-dtype against bf16 attention weights.
  - scores are computed TRANSPOSED: scT[s,h] = sum_d encT[d,s] qtT[d,h]
    with the encT block as the stationary operand.  The [128s, H] result
    is exactly the layout the ctx matmul needs, so no per-tile PE
    transposes of w; exp runs straight PSUM->SBUF; the softmax
    denominator comes from a ones-vector matmul accumulating in PSUM.
  - prologue: dhT + per-jb weight slices DMA first, q/q-tildeT matmuls
    consume each 128-row slice as it lands; q-tildeT is produced by a
    block-diagonal-masked contraction (the zero blocks do the per-head
    masking).
"""

import sys
import numpy as np

for _p in ("/opt/trn_rl_repo",):
    if _p not in sys.path:
        sys.path.insert(0, _p)

import ml_dtypes
import concourse.bass as bass
import concourse.bacc as bacc
import concourse.tile as tile
from concourse import mybir
from concourse.masks import make_identity
from concourse.bass_utils import run_bass_kernel_spmd

B, S, D, H = 16, 4096, 1024, 16
HD = D // H                      # 64
NCORES = 8
BPC = B // NCORES                # 2 batches per core
NJ = D // 128                    # 8 d-blocks
GRP = 4                          # s-tiles (128) per group = 512 positions
SQ = 512                         # encT slab width (= one group)
PACK = 4                         # s-rows packed per partition in e4 tiles

F32 = mybir.dt.float32
BF16 = mybir.dt.bfloat16
FP8 = mybir.dt.float8e4
USE_FP8_C = True          # natural enc (c-tilde rhs) in fp8e4m3; w stays bf16
EDT = FP8 if USE_FP8_C else BF16
EXP_BIAS = -2.0           # exp(s-2); cancels in 1/l


def build_nc(s=S):
    nc = bacc.Bacc(None, target_bir_lowering=False, debug=False)

    dhT_ext = nc.declare_dram_parameter("dhT", [D, BPC], BF16, isOutput=False)
    enc_ext = nc.declare_dram_parameter("enc", [BPC, s, D], EDT, isOutput=False)
    encT_ext = nc.declare_dram_parameter("encT", [BPC, D, s], BF16, isOutput=False)
    wqT_ext = nc.declare_dram_parameter("wqT", [D, D], BF16, isOutput=False)
    wk_ext = nc.declare_dram_parameter("wk", [D, D], BF16, isOutput=False)
    wvT_ext = nc.declare_dram_parameter("wvT", [D, D], BF16, isOutput=False)
    out_ext = nc.declare_dram_parameter("out", [BPC, D], F32, isOutput=True)

    with tile.TileContext(nc) as tc:
        _build(nc, tc, s, dhT_ext, enc_ext, encT_ext, wqT_ext, wk_ext, wvT_ext, out_ext)
    nc.compile()
    return nc


def _build(nc, tc, s, dhT_ext, enc_ext, encT_ext, wqT_ext, wk_ext, wvT_ext, out_ext):
    NT = s // 128                # s-tiles per batch
    NG = NT // GRP               # groups (of 512 positions) per batch
    from contextlib import ExitStack

    ctx = ExitStack()
    with ctx:
        singles = ctx.enter_context(tc.tile_pool(name="singles", bufs=1))
        wjb = ctx.enter_context(tc.tile_pool(name="wjb", bufs=1))
        slabp = ctx.enter_context(tc.tile_pool(name="slabp", bufs=8))
        epool = ctx.enter_context(tc.tile_pool(name="epool", bufs=8))
        sc = ctx.enter_context(tc.tile_pool(name="sc", bufs=2))
        wts = ctx.enter_context(tc.tile_pool(name="wts", bufs=10))
        pp_bf = ctx.enter_context(tc.tile_pool(name="pp_bf", bufs=2, space="PSUM"))
        pp_f32 = ctx.enter_context(tc.tile_pool(name="pp_f32", bufs=2, space="PSUM"))
        pp_l = ctx.enter_context(tc.tile_pool(name="pp_l", bufs=1, space="PSUM"))

        # ---- inputs: dhT first (tiny, unblocks q matmuls), then per-jb
        # slices of WqT and Wk so the prologue pipelines with the DMA.
        dhT_sb = singles.tile([128, NJ, BPC], BF16, tag="dhT")
        nc.sync.dma_start(
            out=dhT_sb, in_=dhT_ext[:, :].rearrange("(jb p) b -> p jb b", p=128)
        )
        wq_jb = []
        wk_jb = []
        for jb in range(NJ):
            t = wjb.tile([128, D], BF16, tag=f"wq{jb}")
            nc.sync.dma_start(out=t, in_=wqT_ext[jb * 128:(jb + 1) * 128, :])
            wq_jb.append(t)
        for jb in range(NJ):
            t = wjb.tile([128, D], BF16, tag=f"wk{jb}")
            nc.sync.dma_start(out=t, in_=wk_ext[jb * 128:(jb + 1) * 128, :])
            wk_jb.append(t)

        # ---- enc streaming: encT slabs (one group wide) and packed fp8
        # natural tiles, both prefetched ahead of use.
        slabs = [[None] * NG for _ in range(BPC)]
        e4s = [[None] * NG for _ in range(BPC)]

        def load_slab(b, g):
            if slabs[b][g] is None:
                t = slabp.tile([128, NJ, SQ], BF16, tag="slab")
                nc.sync.dma_start(
                    out=t,
                    in_=encT_ext[b, :, g * SQ:(g + 1) * SQ].rearrange(
                        "(jb p) t -> p jb t", p=128
                    ),
                )
                slabs[b][g] = t
            return slabs[b][g]

        def load_e4(b, g):
            if e4s[b][g] is None:
                t = epool.tile([128, PACK * D], EDT, tag="e4")
                nc.sync.dma_start(
                    out=t,
                    in_=enc_ext[b, g * 512:(g + 1) * 512, :].rearrange(
                        "(p cls) d -> p (cls d)", p=128
                    ),
                )
                e4s[b][g] = t
            return e4s[b][g]

        for b in range(BPC):
            load_slab(b, 0)
        for b in range(BPC):
            load_e4(b, 0)
        for b in range(BPC):
            load_slab(b, 1)
        for b in range(BPC):
            load_e4(b, 1)

        # ---- constants
        ident = singles.tile([128, 128], BF16)
        make_identity(nc, ident)

        # ---- q[b, i] = sum_d dh[b, d] Wq[i, d]; consume wq slices as they land
        q_ps = pp_f32.tile([BPC, D], F32, tag="pf32")
        for jb in range(NJ):
            for chunk in range(2):
                cs = slice(chunk * 512, (chunk + 1) * 512)
                nc.tensor.matmul(
                    q_ps[:, cs],
                    dhT_sb[:, jb, :],
                    wq_jb[jb][:, cs],
                    start=(jb == 0),
                    stop=(jb == NJ - 1),
                    skip_group_check=True,
                )
        q_sb = singles.tile([BPC, D], BF16, tag="q")
        nc.vector.tensor_copy(out=q_sb, in_=q_ps)

        # qhT: block-diagonal [i, r], r = h*2 + b; qhT[i, r] = q[b, i] iff head(i)==h.
        qhT_sb = singles.tile([128, NJ, 2 * H], BF16, tag="qhT")
        nc.vector.memset(qhT_sb, 0.0)
        for jb in range(NJ):
            ps = pp_bf.tile([128, 128], BF16, tag="ppsum_big")
            nc.tensor.transpose(
                ps[:, 0:BPC], q_sb[:, jb * 128:(jb + 1) * 128], ident[0:BPC, 0:BPC]
            )
            nc.vector.tensor_copy(
                out=qhT_sb[0:64, jb, 4 * jb:4 * jb + 2], in_=ps[0:64, 0:BPC]
            )
            nc.vector.tensor_copy(
                out=qhT_sb[64:128, jb, 4 * jb + 2:4 * jb + 4], in_=ps[64:128, 0:BPC]
            )

        # q-tildeT directly: qtT[d', r] = sum_i Wk[i, d'] qhT[i, r]; the
        # block-diagonal zeros in qhT mask the contraction to head(i)==h(r).
        qtT_all = singles.tile([128, NJ, 2 * H], BF16, tag="qtT_all")
        for db in range(NJ):
            qt_ps = pp_bf.tile([128, 64], F32, tag="ppsum_big")
            for jb_i in range(NJ):
                nc.tensor.matmul(
                    qt_ps[:, 0:2 * H],
                    wk_jb[jb_i][:, db * 128:(db + 1) * 128],
                    qhT_sb[:, jb_i, :],
                    start=(jb_i == 0),
                    stop=(jb_i == NJ - 1),
                )
            nc.vector.tensor_scalar_mul(
                qtT_all[:, db, :], qt_ps[:, 0:2 * H], 1.0 / np.sqrt(HD)
            )
        qtT_b = []
        qtT_v = qtT_all.rearrange("p j (h b) -> p j h b", b=BPC)
        for b in range(BPC):
            t = singles.tile([128, NJ, H], BF16, tag=f"qtT{b}")
            nc.vector.tensor_copy(out=t, in_=qtT_v[:, :, :, b])
            qtT_b.append(t)

        # ---- main streaming loop (single pass, unnormalized-exp softmax)
        cmerged = singles.tile([64, D], BF16, tag="cmerged")
        nc.vector.memset(cmerged, 0.0)
        ebias = singles.tile([128, 1], F32, tag="ebias")
        nc.vector.memset(ebias, EXP_BIAS)
        ones = singles.tile([128, 1], BF16, tag="ones")
        nc.vector.memset(ones, 1.0)

        c_ps = []
        l_ps = []
        for b in range(BPC):
            cp = pp_f32.tile([H, D], F32, tag="pf32")
            c_ps.append(cp)
            lp = pp_l.tile([H, 1], F32, tag=f"l{b}")
            l_ps.append(lp)
        wvT_sb = None
        for g in range(NG):
            for b in range(BPC):
                if g + 2 < NG:
                    load_slab(b, g + 2)
                    load_e4(b, g + 2)
                if g == NG - 2 and b == 0 and wvT_sb is None:
                    wvT_sb = singles.tile([128, NJ, D], BF16, tag="wvT")
                    nc.sync.dma_start(
                        out=wvT_sb,
                        in_=wvT_ext[:, :].rearrange("(jb p) d -> p jb d", p=128),
                    )
                slab = load_slab(b, g)
                e4 = load_e4(b, g)
                for cls in range(GRP):
                    t = g * GRP + cls
                    # scT[s, h] = sum_d encT[d, s] qtT[d, h] for this 128-block
                    scT_ps = pp_bf.tile([128, 64], F32, tag="ppsum_big")
                    for jb in range(NJ):
                        nc.tensor.matmul(
                            scT_ps[:, 0:H],
                            slab[:, jb, cls * 128:(cls + 1) * 128],
                            qtT_b[b][:, jb, :],
                            start=(jb == 0),
                            stop=(jb == NJ - 1),
                        )
                    # unnormalized weights straight from PSUM, already [s, h]
                    wt_t = wts.tile([128, H], BF16, tag="wt")
                    nc.scalar.activation(
                        out=wt_t,
                        in_=scT_ps[:, 0:H],
                        func=mybir.ActivationFunctionType.Exp,
                        bias=ebias,
                    )
                    first = t == 0
                    last = t == NT - 1
                    # softmax denominator: l[h] += sum_s wt[s, h]
                    nc.tensor.matmul(
                        l_ps[b],
                        wt_t,
                        ones,
                        start=first,
                        stop=last,
                        skip_group_check=True,
                    )
                    # c-tilde accumulation against the packed fp8 tile
                    for chunk in range(2):
                        cs = slice(chunk * 512, (chunk + 1) * 512)
                        nc.tensor.matmul(
                            c_ps[b][:, cs],
                            wt_t,
                            e4[:, cls * D + chunk * 512:cls * D + (chunk + 1) * 512],
                            start=first,
                            stop=last,
                            skip_group_check=True,
                        )
        # normalize by 1/sum(exp) while copying out of PSUM
        for b in range(BPC):
            linv = sc.tile([H, 1], F32, tag=f"linv{b}")
            nc.vector.reciprocal(linv, l_ps[b])
            nc.vector.tensor_scalar_mul(
                cmerged[b * 32:b * 32 + H, :], c_ps[b], linv
            )

        # ---- epilogue: cT then per-head final matmuls
        cT_sb = singles.tile([128, NJ, 64], BF16, tag="cT")
        for jb in range(NJ):
            ps = pp_bf.tile([128, 128], BF16, tag="ppsum_big")
            nc.tensor.transpose(
                ps[:, 0:64],
                cmerged[:, jb * 128:(jb + 1) * 128],
                ident[0:64, 0:64],
            )
            nc.vector.tensor_copy(out=cT_sb[:, jb, :], in_=ps[:, 0:64])

        # ctx[b, h*64+j] = sum_d cT[d, b*32+h] WvT[d, h*64+j]
        ctx_ps = pp_f32.tile([BPC, D], F32, tag="pf32")
        cT_v = cT_sb.rearrange("p j (bb h) -> p j bb h", bb=BPC)
        for h in range(H):
            hs = slice(h * HD, (h + 1) * HD)
            for jb in range(NJ):
                nc.tensor.matmul(
                    ctx_ps[:, hs],
                    cT_v[:, jb, :, h],
                    wvT_sb[:, jb, hs],
                    start=(jb == 0),
                    stop=(jb == NJ - 1),
                )
        ob = singles.tile([BPC, D], F32, tag="out_sb")
        nc.vector.tensor_copy(out=ob, in_=ctx_ps)
        nc.sync.dma_start(out=out_ext[:, :], in_=ob)


_NC_CACHE = None


def _get_nc():
    global _NC_CACHE
    if _NC_CACHE is None:
        _NC_CACHE = build_nc()
    return _NC_CACHE


def _sperm(s):
    """Within each 512-block: order positions by class (s mod 4), so scores
    tiles match the packed e4 partition order s = base + 4*p + cls."""
    perm = np.empty(s, dtype=np.int64)
    i = 0
    for base in range(0, s, 512):
        n = min(512, s - base)
        for cls in range(PACK):
            for p in range(n // PACK):
                perm[i] = base + p * PACK + cls
                i += 1
    return perm


def _shard(inputs):
    """Host-side prep: shard batch, cast dtypes, pre-transpose layouts."""
    bf = ml_dtypes.bfloat16
    dh = np.asarray(inputs["decoder_hidden"], dtype=np.float32)
    enc = np.asarray(inputs["encoder_outputs"], dtype=np.float32)
    wqT = np.ascontiguousarray(np.asarray(inputs["Wq"], dtype=np.float32).T).astype(bf)
    wk = np.ascontiguousarray(np.asarray(inputs["Wk"], dtype=np.float32)).astype(bf)
    wvT = np.ascontiguousarray(np.asarray(inputs["Wv"], dtype=np.float32).T).astype(bf)
    cdt = ml_dtypes.float8_e4m3 if USE_FP8_C else bf
    enc_c = enc.astype(cdt)
    perm = _sperm(enc.shape[1])
    in_maps = []
    for c in range(NCORES):
        sl = slice(c * BPC, (c + 1) * BPC)
        dhT = np.ascontiguousarray(dh[sl].T).astype(bf)
        eb = np.ascontiguousarray(enc_c[sl])
        ebT = np.ascontiguousarray(
            enc[sl].astype(bf).transpose(0, 2, 1)[:, :, perm]
        )
        in_maps.append(
            {
                "dhT": dhT,
                "enc": eb,
                "encT": ebT,
                "wqT": wqT,
                "wk": wk,
                "wvT": wvT,
            }
        )
    return in_maps


def _run(inputs, trace=False, **kw):
    global _NC_CACHE
    in_maps = _shard(inputs)
    last_err = None
    for attempt in range(3):
        try:
            nc = _get_nc()
            res = run_bass_kernel_spmd(
                nc, in_maps, core_ids=list(range(NCORES)), trace=trace, **kw
            )
            out = np.concatenate([np.asarray(r["out"]) for r in res.results], axis=0)
            return out.astype(np.float32), res
        except Exception as e:  # transient NRT_EXEC_UNIT_UNRECOVERABLE etc.
            last_err = e
            _NC_CACHE = None  # rebuild the graph fresh on retry
            import time
            time.sleep(2.0)
    raise last_err


def kernel(**inputs):
    out, _ = _run(inputs, trace=False)
    return out
